# revision 1
# baseline (speedup 1.0000x reference)
# Bass/Tile TRN2 kernel for nn_Attn_2130303779132 (general-score attention).
#
# Math: reference computes
#   proj = einsum('sbh,kh->sbk', enc, W) + b        # (S,B,H) huge matmul
#   energies[b,s] = <hidden[b], proj[s,b]>          # (B,S)
#   out = softmax(energies, axis=-1)
# Algebraically:
#   energies[b,s] = sum_h enc[s,b,h] * v[b,h] + (hidden[b]·bias)
# with v = hidden @ W.  The bias term is constant across s, so softmax
# removes it exactly.  The kernel therefore computes v (tiny matmul),
# a batched dot over H against the streamed encoder outputs, and a
# softmax over S — memory bound on reading enc once.
#
# Sharding: data-parallel over batch. 8 cores x 2 batches each.
# W replicated; no collectives.

import numpy as np

import concourse.bacc as bacc
import concourse.bass as bass
import concourse.bass_isa as bass_isa
import concourse.tile as tile
from concourse import library_config, mybir
from concourse.bass_utils import run_bass_kernel_spmd

S, B, H = 4096, 16, 1024
NCORES = 8
BL = B // NCORES          # local batches per core = 2
P = 128                   # partitions
NCHUNK = S // P           # 32 s-chunks of 128
QPT = 2                   # s-chunks per DMA tile
NT = NCHUNK // QPT        # 16 main-loop tiles
KC = H // P               # 8 contraction chunks for v
F32 = mybir.dt.float32

# Engine-mode switches (fallbacks for ops this runtime may not support):
#   REDUCE_MODE: "stt" fused multiply+accum on DVE (1 pass)
#                "act" DVE multiply + ScalarE activation-accumulate reduce
#                "vec" DVE multiply + DVE tensor_reduce (2 DVE passes)
REDUCE_MODE = "stt"
#   BCAST_MODE: "matmul2" selector-matmul straight from the (2,H) v layout
#               | "gpsimd" partition_broadcast | "matmul" ones-matmul on PE
#               (the latter two need the double-transpose row reshuffle)
BCAST_MODE = "matmul2"
#   SMAX_MODE: "gpsimd" partition_all_reduce | "pe" transpose+matmul dance
SMAX_MODE = "gpsimd"
ENC_BUFS = 8
# Number of multiply-reduce chunk jobs (of 64) routed to the otherwise-idle
# GPSIMD engine instead of the DVE (0 = all on DVE).
GP_JOBS = 0
# Ring for the prologue loads (W/hid/eye): "act" = ACT HWDGE ring,
# "sp" = same SP ring as the enc stream (FIFO ahead of it).
W_RING = "sp"


def build_bass(loop_n: int = 1) -> bass.Bass:
    """loop_n > 1 wraps the whole kernel body in an on-device For loop —
    used only for steady-state timing (amortizes RPC/launch overhead)."""
    # Bacc (not plain Bass): its compile() splits multi-wait sync into
    # single-wait instructions and auto-inserts gpsimd library reloads —
    # both required by this walrus build.
    nc = bacc.Bacc("TRN2", target_bir_lowering=False, debug=False,
                   num_devices=NCORES)

    enc = nc.dram_tensor("enc", (S, BL, H), F32, kind="ExternalInput").ap()
    hid = nc.dram_tensor("hid", (BL, H), F32, kind="ExternalInput").ap()
    w = nc.dram_tensor("w", (H, H), F32, kind="ExternalInput").ap()
    eye = nc.dram_tensor("eye", (P, P), F32, kind="ExternalInput").ap()
    selc = nc.dram_tensor("selc", (BL, BL * P), F32, kind="ExternalInput").ap()
    out = nc.dram_tensor("out", (BL, S), F32, kind="ExternalOutput").ap()

    with tile.TileContext(nc) as tc:
        with (
            tc.tile_pool(name="consts", bufs=1) as consts,
            tc.tile_pool(name="wpool", bufs=1) as wpool,
            tc.tile_pool(name="encpool", bufs=ENC_BUFS) as encpool,
            tc.tile_pool(name="scratch", bufs=2) as scratch,
            tc.tile_pool(name="small", bufs=2) as small,
            tc.tile_pool(name="psumc", bufs=1, space="PSUM") as psumc,
            tc.tile_pool(name="psumt", bufs=1, space="PSUM") as psumt,
        ):
            pools = (consts, wpool, encpool, scratch, small, psumc, psumt)

            def body():
                build_body(nc, pools, enc, hid, w, eye, selc, out)

            if loop_n == 1:
                body()
            else:
                with tc.For_i(0, loop_n, 1):
                    body()

    nc.compile()
    return nc


def build_body(nc, pools, enc, hid, w, eye, selc, out):
    consts, wpool, encpool, scratch, small, psumc, psumt = pools

    # Pay the ~6us Q7 library IRAM load up front, overlapped with the W DMAs,
    # instead of right before the first partition_broadcast on the v chain.
    nc.gpsimd.load_library(library_config.mlp)

    ldeng = nc.scalar if W_RING == "act" else nc.sync

    # ---------------- prologue: v = hidden @ W ----------------
    # tiny loads first (they unblock the hidden transposes), then W
    ident = consts.tile([P, P], F32, tag="ident")
    ldeng.dma_start(out=ident, in_=eye)

    hid_sb = consts.tile([BL, H], F32, tag="hid")
    ldeng.dma_start(out=hid_sb, in_=hid)

    w_tiles = []
    for i in range(KC):
        wt = wpool.tile([P, H], F32, tag=f"w{i}", name=f"w{i}")
        ldeng.dma_start(out=wt, in_=w[i * P : (i + 1) * P, :])
        w_tiles.append(wt)

    ones_row = consts.tile([1, P], F32, tag="ones_row")
    nc.vector.memset(ones_row, 1.0)
    ones_col = consts.tile([P, 1], F32, tag="ones_col")
    nc.vector.memset(ones_col, 1.0)

    # hidden^T via PE transposes: hT[k % 128, 2*i + b] = hidden[b, i*128+k%128]
    psum_hT = psumc.tile([P, 2 * KC], F32, tag="hT")
    for i in range(KC):
        nc.tensor.transpose(
            out=psum_hT[:, 2 * i : 2 * i + 2],
            in_=hid_sb[:, i * P : (i + 1) * P],
            identity=ident[0:BL, 0:BL],
        )
    hT_sb = consts.tile([P, 2 * KC], F32, tag="hTsb")
    nc.scalar.copy(out=hT_sb, in_=psum_hT)

    # v = hidden @ W as (2, 1024): out partitions = b (M=2)
    psum_v = psumc.tile([BL, H], F32, tag="v")
    for j in range(H // 512):
        for i in range(KC):
            nc.tensor.matmul(
                out=psum_v[:, j * 512 : (j + 1) * 512],
                lhsT=hT_sb[:, 2 * i : 2 * i + 2],
                rhs=w_tiles[i][:, j * 512 : (j + 1) * 512],
                start=(i == 0),
                stop=(i == KC - 1),
            )
    v_sb = consts.tile([BL, H], F32, tag="vsb")
    nc.scalar.copy(out=v_sb, in_=psum_v)

    if BCAST_MODE == "matmul2":
        # vb[:, b*H:(b+1)*H] = sel_b.T @ v_sb, K=2: sel_b is (2,128) with
        # row b all ones, so the PE replicates v row b to all partitions —
        # no row reshuffle, no gpsimd on the critical path.
        vb = consts.tile([P, BL * H], F32, tag="vb")
        selc_sb = consts.tile([BL, BL * P], F32, tag="selc")
        ldeng.dma_start(out=selc_sb, in_=selc)
        sel = [selc_sb[:, b * P : (b + 1) * P] for b in range(BL)]
        for b in range(BL):
            psum_vb = psumc.tile([P, H], F32, tag="vbp", name=f"psum_vb{b}")
            for j in range(H // 512):
                nc.tensor.matmul(
                    out=psum_vb[:, j * 512 : (j + 1) * 512],
                    lhsT=sel[b],
                    rhs=v_sb[:, j * 512 : (j + 1) * 512],
                    start=True,
                    stop=True,
                )
            nc.scalar.copy(out=vb[:, b * H : (b + 1) * H], in_=psum_vb)

    # ---------------- main loop: energies ----------------
    # E[b][p, c] = sum_h enc[c*128+p, b, h] * v[b, h]
    E = [
        consts.tile([P, NCHUNK], F32, tag=f"E{b}", name=f"E{b}")
        for b in range(BL)
    ]
    enc_r = enc.rearrange("(n q p) b h -> n p q b h", q=QPT, p=P)
    njobs = NT * QPT * BL
    gp_every = njobs // GP_JOBS if GP_JOBS else njobs + 1
    job = 0
    for t in range(NT):
        et = encpool.tile([P, QPT, BL, H], F32, tag="enc")
        nc.sync.dma_start(out=et, in_=enc_r[t])
        for q in range(QPT):
            for b in range(BL):
                c = t * QPT + q
                eslice = et[:, q, b, :]
                vslice = vb[:, b * H : (b + 1) * H]
                eout = E[b][:, c : c + 1]
                job += 1
                if GP_JOBS and job % gp_every == 0:
                    prodg = scratch.tile([P, H], F32, tag="prodg",
                                         name="prodg")
                    nc.gpsimd.scalar_tensor_tensor(
                        out=prodg, in0=eslice, scalar=1.0, in1=vslice,
                        op0=mybir.AluOpType.mult, op1=mybir.AluOpType.mult,
                        accum_out=eout,
                    )
                elif REDUCE_MODE == "stt":
                    prod = scratch.tile([P, H], F32, tag="prod")
                    nc.vector.scalar_tensor_tensor(
                        out=prod, in0=eslice, scalar=1.0, in1=vslice,
                        op0=mybir.AluOpType.mult, op1=mybir.AluOpType.mult,
                        accum_out=eout,
                    )
                elif REDUCE_MODE == "act":
                    prod = scratch.tile([P, H], F32, tag="prod")
                    prod2 = scratch.tile([P, H], F32, tag="prod2")
                    nc.vector.tensor_mul(prod, eslice, vslice)
                    nc.scalar.activation(
                        out=prod2, in_=prod,
                        func=mybir.ActivationFunctionType.Copy,
                        accum_out=eout,
                    )
                else:  # "vec"
                    prod = scratch.tile([P, H], F32, tag="prod")
                    nc.vector.tensor_mul(prod, eslice, vslice)
                    nc.vector.tensor_reduce(
                        out=eout, in_=prod, axis=mybir.AxisListType.X,
                        op=mybir.AluOpType.add,
                    )

    # ---------------- softmax over S per local batch ----------------
    for b in range(BL):
        negm = small.tile([P, 1], F32, tag="negm")
        if SMAX_MODE == "gpsimd":
            m_all = small.tile([P, NCHUNK], F32, tag="mall")
            nc.gpsimd.partition_all_reduce(
                out_ap=m_all, in_ap=E[b], channels=P,
                reduce_op=bass_isa.ReduceOp.max,
            )
            nc.vector.tensor_reduce(
                out=negm, in_=m_all, axis=mybir.AxisListType.X,
                op=mybir.AluOpType.max, negate=True,
            )
        else:  # "pe": transpose E, reduce, transpose, reduce, broadcast
            psum_Et = psumt.tile([NCHUNK, P], F32, tag="pT", name="psum_Et")
            nc.tensor.transpose(out=psum_Et, in_=E[b], identity=ident)
            Et = small.tile([NCHUNK, P], F32, tag="Et")
            nc.scalar.copy(out=Et, in_=psum_Et)
            cmax = small.tile([NCHUNK, 1], F32, tag="cmax")
            nc.vector.tensor_reduce(
                out=cmax, in_=Et, axis=mybir.AxisListType.X,
                op=mybir.AluOpType.max,
            )
            psum_cmaxT = psumt.tile([1, NCHUNK], F32, tag="tiny",
                                    name="psum_cmaxT")
            nc.tensor.transpose(
                out=psum_cmaxT, in_=cmax, identity=ident[0:NCHUNK, 0:NCHUNK]
            )
            cmaxT = small.tile([1, NCHUNK], F32, tag="cmaxT")
            nc.scalar.copy(out=cmaxT, in_=psum_cmaxT)
            negm1 = small.tile([1, 1], F32, tag="negm1")
            nc.vector.tensor_reduce(
                out=negm1, in_=cmaxT, axis=mybir.AxisListType.X,
                op=mybir.AluOpType.max, negate=True,
            )
            psum_negm = psumt.tile([P, 1], F32, tag="tiny", name="psum_negm")
            nc.tensor.matmul(out=psum_negm, lhsT=ones_row, rhs=negm1,
                             start=True, stop=True)
            nc.scalar.copy(out=negm, in_=psum_negm)

        eexp = small.tile([P, NCHUNK], F32, tag="eexp")
        nc.scalar.activation(
            out=eexp, in_=E[b],
            func=mybir.ActivationFunctionType.Exp,
            bias=negm, scale=1.0,
        )
        rowsum = small.tile([P, 1], F32, tag="rowsum")
        nc.vector.tensor_reduce(
            out=rowsum, in_=eexp, axis=mybir.AxisListType.X,
            op=mybir.AluOpType.add,
        )
        rtot = small.tile([P, 1], F32, tag="rtot")
        if SMAX_MODE == "gpsimd":
            tot = small.tile([P, 1], F32, tag="tot")
            nc.gpsimd.partition_all_reduce(
                out_ap=tot, in_ap=rowsum, channels=P,
                reduce_op=bass_isa.ReduceOp.add,
            )
            nc.vector.reciprocal(out=rtot, in_=tot)
        else:
            psum_tot = psumt.tile([1, 1], F32, tag="tiny", name="psum_tot")
            nc.tensor.matmul(out=psum_tot, lhsT=rowsum, rhs=ones_col,
                             start=True, stop=True)
            tot1 = small.tile([1, 1], F32, tag="tot1")
            nc.scalar.copy(out=tot1, in_=psum_tot)
            rtot1 = small.tile([1, 1], F32, tag="rtot1")
            nc.vector.reciprocal(out=rtot1, in_=tot1)
            psum_rtot = psumt.tile([P, 1], F32, tag="tiny", name="psum_rtot")
            nc.tensor.matmul(out=psum_rtot, lhsT=ones_row, rhs=rtot1,
                             start=True, stop=True)
            nc.scalar.copy(out=rtot, in_=psum_rtot)

        probs = small.tile([P, NCHUNK], F32, tag="probs")
        nc.vector.tensor_scalar_mul(out=probs, in0=eexp, scalar1=rtot)

        pT = psumt.tile([NCHUNK, P], F32, tag="pT", name="pT")
        nc.tensor.transpose(out=pT, in_=probs, identity=ident)
        pT_sb = small.tile([NCHUNK, P], F32, tag="pTsb")
        nc.scalar.copy(out=pT_sb, in_=pT)
        nc.sync.dma_start(
            out=out[b].rearrange("(c p) -> c p", p=P), in_=pT_sb
        )


_NC_CACHE = None


def _get_nc() -> bass.Bass:
    global _NC_CACHE
    if _NC_CACHE is None:
        _NC_CACHE = build_bass()
    return _NC_CACHE


def make_in_maps(hidden, encoder_outputs, W):
    hidden = np.asarray(hidden, dtype=np.float32)
    encoder_outputs = np.asarray(encoder_outputs, dtype=np.float32)
    W = np.ascontiguousarray(np.asarray(W, dtype=np.float32))
    eye = np.eye(P, dtype=np.float32)
    selc = np.zeros((BL, BL * P), dtype=np.float32)
    for b in range(BL):
        selc[b, b * P : (b + 1) * P] = 1.0
    in_maps = []
    for c in range(NCORES):
        in_maps.append(
            {
                "enc": np.ascontiguousarray(
                    encoder_outputs[:, c * BL : (c + 1) * BL, :]
                ),
                "hid": np.ascontiguousarray(hidden[0, c * BL : (c + 1) * BL, :]),
                "w": W,
                "eye": eye,
                "selc": selc,
            }
        )
    return in_maps


def kernel(hidden, encoder_outputs, W, b, **run_kwargs):
    # `b` (the nn.Linear bias) shifts every energy row by a per-batch
    # constant, which softmax cancels exactly — unused on device.
    nc = _get_nc()
    in_maps = make_in_maps(hidden, encoder_outputs, W)
    res = run_bass_kernel_spmd(
        nc, in_maps, core_ids=list(range(NCORES)), **run_kwargs
    )
    outs = [r["out"] for r in res.results]
    full = np.concatenate(outs, axis=0)  # (16, 4096)
    return full.reshape(B, 1, S).astype(np.float32)



# revision 21
# speedup vs baseline: 1.9312x; 1.9312x over previous
# Bass/Tile TRN2 kernel for nn_Attn_2130303779132 (general-score attention).
#
# Math: reference computes
#   proj = einsum('sbh,kh->sbk', enc, W) + b        # (S,B,H) huge matmul
#   energies[b,s] = <hidden[b], proj[s,b]>          # (B,S)
#   out = softmax(energies, axis=-1)
# Algebraically:
#   energies[b,s] = sum_h enc[s,b,h] * v[b,h] + (hidden[b]·bias)
# with v = hidden @ W.  The bias term is constant across s, so softmax
# removes it exactly.
#
# v2 design (PE-stationary, fp16 stream):
#   * Host pre-transposes enc to (BL, KC, 128, S) fp16 per core — h on
#     partitions, s on the free axis, halving HBM traffic vs f32 and
#     giving perfectly contiguous 8KB-per-partition DMA descriptors.
#   * vT (v transposed to [128, KC*BL], h on partitions) comes from
#     64 tiny PE matmuls against host-uploaded W^T fp16 — no big f32
#     matmul on the critical path.
#   * E[b][s_chunk] = one PE matmul per (b, h-chunk, s-chunk):
#     lhsT = enc tile [128h, 128s], rhs = vT column [128, 1], PSUM-
#     accumulated over the 8 h-chunks.  E lands partition-major
#     ([128, 32] per batch) so the softmax needs no gpsimd.
#   * Softmax reductions across partitions via PE transpose + ones-
#     matmul; exp on ACT straight out of PSUM.
#
# Sharding: data-parallel over batch. 8 cores x 2 batches each.
# W replicated; no collectives.

import numpy as np

import concourse.bacc as bacc
import concourse.bass as bass
import concourse.tile as tile
from concourse import mybir
from concourse.bass_utils import run_bass_kernel_spmd

S, B, H = 4096, 16, 1024
NCORES = 8
BL = B // NCORES          # local batches per core = 2
P = 128                   # partitions
KC = H // P               # 8 h-chunks
NCHUNK = S // P           # 32 s-chunks of 128
KPT = 2                   # h-chunks per DMA tile
NT = BL * KC // KPT       # 8 main-loop DMA tiles (4 per batch)
F32 = mybir.dt.float32
F16 = mybir.dt.float16


def build_bass(loop_n: int = 1) -> bass.Bass:
    """loop_n > 1 wraps the whole kernel body in an on-device For loop —
    used only for steady-state timing (amortizes RPC/launch overhead)."""
    nc = bacc.Bacc("TRN2", target_bir_lowering=False, debug=False,
                   num_devices=NCORES)

    enc = nc.dram_tensor("enc", (BL, KC, P, S), F16, kind="ExternalInput").ap()
    hid = nc.dram_tensor("hid", (BL, H), F16, kind="ExternalInput").ap()
    w = nc.dram_tensor("w", (H, H), F16, kind="ExternalInput").ap()
    eye = nc.dram_tensor("eye", (P, P), F32, kind="ExternalInput").ap()
    out = nc.dram_tensor("out", (BL, S), F32, kind="ExternalOutput").ap()

    with tile.TileContext(nc) as tc:
        with (
            tc.tile_pool(name="consts", bufs=1) as consts,
            tc.tile_pool(name="wpool", bufs=1) as wpool,
            tc.tile_pool(name="encpool", bufs=3) as encpool,
            tc.tile_pool(name="small", bufs=2) as small,
            tc.tile_pool(name="psumc", bufs=1, space="PSUM") as psumc,
            tc.tile_pool(name="psume", bufs=1, space="PSUM") as psume,
            tc.tile_pool(name="psumt", bufs=1, space="PSUM") as psumt,
        ):
            pools = (consts, wpool, encpool, small, psumc, psume, psumt)

            def body():
                build_body(nc, pools, enc, hid, w, eye, out)

            if loop_n == 1:
                body()
            else:
                with tc.For_i(0, loop_n, 1):
                    body()

    nc.compile()
    return nc


def build_body(nc, pools, enc, hid, w, eye, out):
    consts, wpool, encpool, small, psumc, psume, psumt = pools

    # ---------------- prologue: vT = (hidden @ W)^T as [128, KC*BL] ----
    ident = consts.tile([P, P], F32, tag="ident")
    nc.scalar.dma_start(out=ident, in_=eye)
    ident16 = consts.tile([P, P], F16, tag="ident16")
    nc.scalar.copy(out=ident16, in_=ident)

    hid_sb = consts.tile([BL, H], F16, tag="hid")
    nc.scalar.dma_start(out=hid_sb, in_=hid)

    w_tiles = []
    for i in range(KC):
        w_t = wpool.tile([P, H], F16, tag=f"w{i}", name=f"w{i}")
        nc.scalar.dma_start(out=w_t, in_=w[i * P: (i + 1) * P, :])
        w_tiles.append(w_t)

    ones_row = consts.tile([1, P], F32, tag="ones_row")
    nc.vector.memset(ones_row, 1.0)
    ones_col = consts.tile([P, 1], F32, tag="ones_col")
    nc.vector.memset(ones_col, 1.0)
    # Preload the Exp table while the PE prologue runs so the first real
    # softmax doesn't eat the LoadActFuncSet latency.
    actwarm = consts.tile([1, 1], F32, tag="actwarm")
    nc.scalar.activation(out=actwarm, in_=ones_row[:, 0:1],
                         func=mybir.ActivationFunctionType.Exp)

    # hidden^T via PE transposes: hT[:, 2i + b] = hidden[b, i*128 : ...]
    psum_hT = psumc.tile([P, BL * KC], F16, tag="hT")
    for i in range(KC):
        nc.tensor.transpose(
            out=psum_hT[:, BL * i: BL * i + BL],
            in_=hid_sb[:, i * P: (i + 1) * P],
            identity=ident16[0:BL, 0:BL],
        )
    hT_sb = consts.tile([P, BL * KC], F16, tag="hTsb")
    nc.scalar.copy(out=hT_sb, in_=psum_hT)

    # vT[:, BL*j + b] = v[b, j*128 : (j+1)*128] where v = hidden @ W
    # (v[b,h] = sum_k hid[b,k] W[k,h]):
    # out[h128, b] = sum_k W[k, h] * hidT[k, b], accumulated over the
    # 8 k-chunks.  W natural layout gives the lhsT block directly:
    # w_tiles[i][p, j*128+c] = W[i*128+p, j*128+c].
    psum_vT = psumc.tile([P, BL * KC], F32, tag="vT")
    for j in range(KC):
        for i in range(KC):
            nc.tensor.matmul(
                out=psum_vT[:, BL * j: BL * j + BL],
                lhsT=w_tiles[i][:, j * P: (j + 1) * P],
                rhs=hT_sb[:, BL * i: BL * i + BL],
                start=(i == 0),
                stop=(i == KC - 1),
            )
    vT16 = consts.tile([P, BL * KC], F16, tag="vT16")
    nc.scalar.copy(out=vT16, in_=psum_vT)

    # ---------------- main loop: E via PE-stationary matmuls ----------
    # E[b][p, sc] = sum_h enc[s=sc*128+p, b, h] * v[b, h]
    psum_E = [
        psume.tile([P, NCHUNK], F32, tag=f"E{b}", name=f"E{b}")
        for b in range(BL)
    ]
    # One DMA tile per k-group; the last two groups are single k-chunks
    # (1 MB) so only 32 matmuls depend on the final transfer — the PE
    # consumes each tile as it lands.  All of a batch's matmuls form ONE
    # PSUM accumulation group (start zeroes the whole bank lazily, so
    # each column gets zero-init on first touch; stop only at the end).
    K_GROUPS = [(0, 1), (2, 3), (4, 5), (6,), (7,)]
    for b in range(BL):
        for gi, ks in enumerate(K_GROUPS):
            et = encpool.tile([P, len(ks), S], F16, tag=f"enc{len(ks)}")
            nc.sync.dma_start(
                out=et,
                in_=enc[b, ks[0]: ks[-1] + 1].rearrange("k p s -> p k s"),
            )
            for sc in range(NCHUNK):
                for q, k in enumerate(ks):
                    nc.tensor.matmul(
                        out=psum_E[b][:, sc: sc + 1],
                        lhsT=et[:, q, sc * P: (sc + 1) * P],
                        rhs=vT16[:, BL * k + b: BL * k + b + 1],
                        start=(gi == 0 and sc == 0 and q == 0),
                        stop=(gi == len(K_GROUPS) - 1
                              and sc == NCHUNK - 1 and q == len(ks) - 1),
                    )
        softmax_store(nc, pools, psum_E[b], out, b, ident,
                      ones_row, ones_col)


def softmax_store(nc, pools, psum_Eb, out, b, ident, ones_row, ones_col):
    consts, wpool, encpool, small, psumc, psume, psumt = pools

    # global max over S: free-dim max on DVE, partition max via PE
    # transpose + free-dim max, then broadcast back to [128, 1].
    rmax = small.tile([P, 1], F32, tag="rmax")
    nc.vector.tensor_reduce(
        out=rmax, in_=psum_Eb, axis=mybir.AxisListType.X,
        op=mybir.AluOpType.max,
    )
    psum_rt = psumt.tile([1, P], F32, tag="rt", name="psum_rt")
    nc.tensor.transpose(out=psum_rt, in_=rmax, identity=ident)
    negm1 = small.tile([1, 1], F32, tag="negm1")
    nc.vector.tensor_reduce(
        out=negm1, in_=psum_rt, axis=mybir.AxisListType.X,
        op=mybir.AluOpType.max, negate=True,
    )
    psum_negm = psumt.tile([P, 1], F32, tag="bc", name="psum_negm")
    nc.tensor.matmul(out=psum_negm, lhsT=ones_row, rhs=negm1,
                     start=True, stop=True)
    negm = small.tile([P, 1], F32, tag="negm")
    nc.scalar.copy(out=negm, in_=psum_negm)

    # exp(E - max) straight out of PSUM on ACT
    eexp = small.tile([P, NCHUNK], F32, tag="eexp")
    nc.scalar.activation(
        out=eexp, in_=psum_Eb,
        func=mybir.ActivationFunctionType.Exp,
        bias=negm, scale=1.0,
    )
    # transpose the unnormalized numerator early; the 1/Z scale lands in
    # the final ACT copy as a per-partition scalar.
    psum_eT = psumt.tile([NCHUNK, P], F32, tag="pT", name="psum_eT")
    nc.tensor.transpose(out=psum_eT, in_=eexp, identity=ident)

    # total = sum over S: free-dim sum on DVE, partition sum via ones-
    # matmul (contracts the partition axis), then reciprocal.
    rowsum = small.tile([P, 1], F32, tag="rowsum")
    nc.vector.tensor_reduce(
        out=rowsum, in_=eexp, axis=mybir.AxisListType.X,
        op=mybir.AluOpType.add,
    )
    # reuse dead PSUM slices (rt after negm1, bc after the negm copy)
    # for the two scalar broadcasts — keeps psumt at 3 banks.
    psum_tot = psum_rt[0:1, 0:1]
    nc.tensor.matmul(out=psum_tot, lhsT=rowsum, rhs=ones_col,
                     start=True, stop=True)
    rtot1 = small.tile([1, 1], F32, tag="rtot1")
    nc.vector.reciprocal(out=rtot1, in_=psum_tot)
    psum_rtot = psum_negm[0:NCHUNK, 0:1]
    nc.tensor.matmul(out=psum_rtot, lhsT=ones_row[:, 0:NCHUNK], rhs=rtot1,
                     start=True, stop=True)
    rtot32 = small.tile([NCHUNK, 1], F32, tag="rtot32")
    nc.scalar.copy(out=rtot32, in_=psum_rtot)

    # out[b][sc*128 + p] = eexp[p, sc] / Z: scaled copy of the transpose
    pT_sb = small.tile([NCHUNK, P], F32, tag="pTsb")
    nc.scalar.activation(
        out=pT_sb, in_=psum_eT,
        func=mybir.ActivationFunctionType.Copy,
        scale=rtot32,
    )
    nc.scalar.dma_start(
        out=out[b].rearrange("(c p) -> c p", p=P), in_=pT_sb
    )


_NC_CACHE = None


def _get_nc() -> bass.Bass:
    global _NC_CACHE
    if _NC_CACHE is None:
        _NC_CACHE = build_bass()
    return _NC_CACHE


def make_in_maps(hidden, encoder_outputs, W):
    hidden = np.asarray(hidden, dtype=np.float32)
    encoder_outputs = np.asarray(encoder_outputs, dtype=np.float32)
    W = np.asarray(W, dtype=np.float32)
    # (S, B, H) -> (B, H, S) fp16, h-major so each core's slice is one
    # contiguous block of tiles [128h, S].
    encT = np.ascontiguousarray(
        encoder_outputs.transpose(1, 2, 0)
    ).astype(np.float16)
    w16 = np.ascontiguousarray(W).astype(np.float16)
    hid16 = hidden[0].astype(np.float16)
    eye = np.eye(P, dtype=np.float32)
    in_maps = []
    for c in range(NCORES):
        in_maps.append(
            {
                "enc": encT[c * BL: (c + 1) * BL].reshape(BL, KC, P, S),
                "hid": np.ascontiguousarray(hid16[c * BL: (c + 1) * BL]),
                "w": w16,
                "eye": eye,
            }
        )
    return in_maps


def kernel(hidden, encoder_outputs, W, b, **run_kwargs):
    # `b` (the nn.Linear bias) shifts every energy row by a per-batch
    # constant, which softmax cancels exactly — unused on device.
    nc = _get_nc()
    in_maps = make_in_maps(hidden, encoder_outputs, W)
    res = run_bass_kernel_spmd(
        nc, in_maps, core_ids=list(range(NCORES)), **run_kwargs
    )
    outs = [r["out"] for r in res.results]
    full = np.concatenate(outs, axis=0)  # (16, 4096)
    return full.reshape(B, 1, S).astype(np.float32)
